# revision 8
# baseline (speedup 1.0000x reference)
"""Trainium2 Bass kernel for a 6-layer GRU stack (T=1, B=1, H=2048) + LayerNorm +
Linear with residual, tensor-parallel over 8 NeuronCores.

Contract: kernel(**inputs) takes the FULL unsharded inputs (as produced by the
reference setup_inputs) and returns the full outputs (pred [1,2048],
new_hidden [6,1,2048]).

Strategy (hardcoded for L=6, H=2048, 8 cores):
  - Shard each layer's 3H=6144 gate rows: core k owns rows [k*256,(k+1)*256) of
    each of the r/z/n gate blocks (768 rows/core/matrix).
  - Host-side: slice + transpose weights to W^T, chunk the contraction dim into
    16 chunks of 128 (chunk n holds contraction elements {16p+n}), cast bf16.
  - Device: GEMV = 16 matmuls with a [128,1] slice of the hidden-state
    stationary tile ([128,16] = h.reshape(128,16)) against [128,768] moving
    weight tiles, accumulating in PSUM.  Biases folded in via 1-contraction
    matmuls against a ones stationary.
  - Gate nonlinearities on [1,768]-ish tiles (ScalarE sigmoid/tanh, VectorE TT).
  - Per layer: AllGather of the 256-element h_new chunk (bf16) via DRAM bounce
    to rebuild the full 2048 hidden state on every core.
  - Tail: LayerNorm computed redundantly on every core from the final gathered
    h (partition-sum / broadcast via ones-matmuls), then a row-sharded Linear
    (256 out rows/core) + residual; host concatenates the 8 chunks.
"""
import numpy as np
import ml_dtypes

L = 6
H = 2048
NCORES = 8
S = H // NCORES          # 256  per-core hidden chunk
G = 3 * S                # 768  gate rows per core
NCH = H // 128           # 16   contraction chunks
LN_EPS = 1e-5

BF16 = ml_dtypes.bfloat16

_cache = {}


def _build_nc():
    import concourse.bacc as bacc
    import concourse.mybir as mybir
    import concourse.tile as tile

    dt = mybir.dt
    AF = mybir.ActivationFunctionType
    ALU = mybir.AluOpType

    nc = bacc.Bacc("TRN2", target_bir_lowering=False, debug=False,
                   num_devices=NCORES)

    wi = nc.dram_tensor("wi", [L, 128, NCH, G], dt.bfloat16, kind="ExternalInput")
    wh = nc.dram_tensor("wh", [L, 128, NCH, G], dt.bfloat16, kind="ExternalInput")
    lw = nc.dram_tensor("lw", [128, NCH, S], dt.bfloat16, kind="ExternalInput")
    xst = nc.dram_tensor("xst", [128, NCH], dt.bfloat16, kind="ExternalInput")
    hst = nc.dram_tensor("hst", [128, L, NCH], dt.bfloat16, kind="ExternalInput")
    hch = nc.dram_tensor("hch", [1, L * S], dt.float32, kind="ExternalInput")
    brz = nc.dram_tensor("brz", [1, L * 2 * S], dt.bfloat16, kind="ExternalInput")
    bin_ = nc.dram_tensor("bin", [1, L * S], dt.bfloat16, kind="ExternalInput")
    bhn = nc.dram_tensor("bhn", [1, L * S], dt.bfloat16, kind="ExternalInput")
    lb = nc.dram_tensor("lb", [1, S], dt.float32, kind="ExternalInput")
    xres = nc.dram_tensor("xres", [1, S], dt.float32, kind="ExternalInput")
    gam = nc.dram_tensor("gam", [128, NCH], dt.float32, kind="ExternalInput")
    bet = nc.dram_tensor("bet", [128, NCH], dt.float32, kind="ExternalInput")
    ones = nc.dram_tensor("ones", [128, 128], dt.float32, kind="ExternalInput")

    out_h = nc.dram_tensor("out_h", [L, S], dt.float32, kind="ExternalOutput")
    out_p = nc.dram_tensor("out_p", [1, S], dt.float32, kind="ExternalOutput")

    RG = [list(range(NCORES))]

    with tile.TileContext(nc) as tc:
        with tc.tile_pool(name="wpool", bufs=2) as wpool, \
             tc.tile_pool(name="spool", bufs=2) as spool, \
             tc.tile_pool(name="cpool", bufs=1) as cpool, \
             tc.tile_pool(name="psum", bufs=2, space="PSUM") as pp, \
             tc.tile_pool(name="ptail", bufs=2, space="PSUM") as pt, \
             tc.tile_pool(name="dram", bufs=2, space="DRAM") as dpool:

            # ---- constants / small inputs (loaded once) ----
            xst_sb = cpool.tile([128, NCH], dt.bfloat16)
            nc.sync.dma_start(out=xst_sb[:], in_=xst.ap())
            hst_sb = cpool.tile([128, L, NCH], dt.bfloat16)
            nc.sync.dma_start(out=hst_sb[:], in_=hst.ap())
            hch_sb = cpool.tile([1, L * S], dt.float32)
            nc.sync.dma_start(out=hch_sb[:], in_=hch.ap())
            brz_sb = cpool.tile([1, L * 2 * S], dt.bfloat16)
            nc.sync.dma_start(out=brz_sb[:], in_=brz.ap())
            bin_sb = cpool.tile([1, L * S], dt.bfloat16)
            nc.sync.dma_start(out=bin_sb[:], in_=bin_.ap())
            bhn_sb = cpool.tile([1, L * S], dt.bfloat16)
            nc.sync.dma_start(out=bhn_sb[:], in_=bhn.ap())
            lb_sb = cpool.tile([1, S], dt.float32)
            nc.sync.dma_start(out=lb_sb[:], in_=lb.ap())
            xres_sb = cpool.tile([1, S], dt.float32)
            nc.sync.dma_start(out=xres_sb[:], in_=xres.ap())
            gam_sb = cpool.tile([128, NCH], dt.float32)
            nc.sync.dma_start(out=gam_sb[:], in_=gam.ap())
            bet_sb = cpool.tile([128, NCH], dt.float32)
            nc.sync.dma_start(out=bet_sb[:], in_=bet.ap())
            ones_sb = cpool.tile([128, 128], dt.float32)
            nc.sync.dma_start(out=ones_sb[:], in_=ones.ap())
            onesb_sb = cpool.tile([1, 16], dt.bfloat16)
            nc.gpsimd.dma_start(out=onesb_sb[:], in_=ones.ap()[0:1, 0:16])
            lw_sb = cpool.tile([128, NCH, S], dt.bfloat16)
            nc.sync.dma_start(out=lw_sb[:], in_=lw.ap())

            one1 = onesb_sb[0:1, 0:1]         # [1,1] bf16 bias-matmul stationary
            one1f = ones_sb[0:1, 0:1]         # [1,1] fp32
            ones_col = ones_sb[:, 0:1]        # [128,1] partition-sum stationary
            ones_row = ones_sb[0:1, :]        # [1,128] broadcast stationary

            prev_st = xst_sb                  # layer-0 gi stationary = x_t

            psA = {}   # [1,512] psum: r|z preactivation (gi+gh+biases)
            psB = {}   # [1,512] psum: gi_n (cols 0:S) | gh_n (cols S:2S)
            wis = {}
            whs = {}

            # warmup collective: absorbs ncfw first-call cost + aligns cores
            # while the first weight DMAs stream.
            wu_in = dpool.tile([1, 8], dt.float32, tag="wuin")
            nc.sync.dma_start(out=wu_in[:], in_=ones.ap()[0:1, 0:8])
            wu_out = dpool.tile([NCORES, 8], dt.float32, tag="wuout",
                                addr_space="Shared")
            nc.gpsimd.collective_compute(
                "AllGather", ALU.bypass, replica_groups=RG,
                ins=[wu_in[:]], outs=[wu_out[:]],
            )

            NH = NCH // 2   # half-layer granularity

            def emit_wdma(l):
                # half-layer weight DMAs on the ACT HWDGE ring
                wia = wpool.tile([128, NH, G], dt.bfloat16, tag="wi", bufs=7)
                nc.scalar.dma_start(out=wia[:], in_=wi.ap()[l, :, 0:NH])
                wib = wpool.tile([128, NH, G], dt.bfloat16, tag="wi", bufs=7)
                nc.scalar.dma_start(out=wib[:], in_=wi.ap()[l, :, NH:NCH])
                wis[l] = (wia, wib)
                wha = wpool.tile([128, NH, G], dt.bfloat16, tag="wh", bufs=6)
                nc.scalar.dma_start(out=wha[:], in_=wh.ap()[l, :, 0:NH])
                whb = wpool.tile([128, NH, G], dt.bfloat16, tag="wh", bufs=6)
                nc.scalar.dma_start(out=whb[:], in_=wh.ap()[l, :, NH:NCH])
                whs[l] = (wha, whb)

            def emit_wgh(l):
                wha, whb = whs[l]
                A = pp.tile([1, 2 * S], dt.float32, tag="A", bufs=3)
                B = pp.tile([1, 2 * S], dt.float32, tag="B", bufs=3)
                psA[l], psB[l] = A, B
                nc.tensor.matmul(A[:], one1,
                                 brz_sb[:, l * 2 * S:(l + 1) * 2 * S],
                                 start=True, stop=False)
                nc.tensor.matmul(B[:, 0:S], one1, bin_sb[:, l * S:(l + 1) * S],
                                 start=True, stop=False)
                nc.tensor.matmul(B[:, S:2 * S], one1, bhn_sb[:, l * S:(l + 1) * S],
                                 start=True, stop=False)
                for n in range(NCH):
                    st = hst_sb[:, l, n:n + 1]
                    w = wha if n < NH else whb
                    nc.tensor.matmul(A[:], st, w[:, n % NH, 0:2 * S],
                                     start=False, stop=False)
                    nc.tensor.matmul(B[:, S:2 * S], st, w[:, n % NH, 2 * S:G],
                                     start=False, stop=(n == NCH - 1))

            def emit_gi(l):
                A, B = psA[l], psB[l]
                wia, wib = wis[l]
                for n in range(NCH):       # A first: sigmoid starts earlier
                    st = prev_st[:, n:n + 1]
                    w = wia if n < NH else wib
                    nc.tensor.matmul(A[:], st, w[:, n % NH, 0:2 * S],
                                     start=False, stop=(n == NCH - 1))
                for n in range(NCH):
                    st = prev_st[:, n:n + 1]
                    w = wia if n < NH else wib
                    nc.tensor.matmul(B[:, 0:S], st, w[:, n % NH, 2 * S:G],
                                     start=False, stop=(n == NCH - 1))

            emit_wdma(0)
            emit_wdma(1)
            emit_wdma(2)
            # PE warmup: keep HAM busy while the first weights stream in
            ps_warm = pp.tile([1, NCH], dt.float32, tag="warm", bufs=1)
            for i in range(24):
                nc.tensor.matmul(ps_warm[:], xst_sb[:, 0:1], xst_sb[:, :],
                                 start=(i == 0), stop=(i == 23))
            emit_wgh(0)
            for l in range(L):
                if l + 3 < L:
                    emit_wdma(l + 3)
                if l + 1 < L:
                    emit_wgh(l + 1)
                emit_gi(l)
                A, B = psA[l], psB[l]

                # ---- gates ----
                sig = spool.tile([1, 2 * S], dt.float32, tag="sig")
                nc.scalar.activation(sig[:], A[:], AF.Sigmoid)
                t_ = spool.tile([1, S], dt.float32, tag="t")
                nc.vector.tensor_mul(t_[:], sig[:, 0:S], B[:, S:2 * S])
                u_ = spool.tile([1, S], dt.float32, tag="u")
                nc.vector.tensor_add(u_[:], B[:, 0:S], t_[:])
                nn_ = spool.tile([1, S], dt.float32, tag="nn")
                nc.scalar.activation(nn_[:], u_[:], AF.Tanh)
                d_ = spool.tile([1, S], dt.float32, tag="d")
                nc.vector.tensor_sub(d_[:], hch_sb[:, l * S:(l + 1) * S], nn_[:])
                zd = spool.tile([1, S], dt.float32, tag="zd")
                nc.vector.tensor_mul(zd[:], sig[:, S:2 * S], d_[:])
                hnew = spool.tile([1, S], dt.bfloat16, tag="hnew")
                nc.vector.tensor_add(hnew[:], nn_[:], zd[:])

                # fp32 hidden-state output (off critical path, SWDGE cast)
                nc.gpsimd.dma_start(out=out_h.ap()[l:l + 1, :], in_=hnew[:])

                # ---- exchange: AllGather, reload as stationary ----
                ag_in = dpool.tile([1, S], dt.bfloat16, tag="agin")
                nc.sync.dma_start(out=ag_in[:], in_=hnew[:])
                ag_out = dpool.tile([128, NCH], dt.bfloat16, tag="agout",
                                    addr_space="Shared")
                nc.gpsimd.collective_compute(
                    "AllGather", ALU.bypass, replica_groups=RG,
                    ins=[ag_in[:]], outs=[ag_out[:]],
                )
                h_st = spool.tile([128, NCH], dt.bfloat16, tag="hstat")
                nc.sync.dma_start(out=h_st[:], in_=ag_out[:])
                prev_st = h_st

            # ---- LayerNorm on the gathered top hidden state ----
            stats = spool.tile([128, 2], dt.float32)
            nc.vector.reduce_sum(stats[:, 0:1], prev_st[:], axis=mybir.AxisListType.X)
            sq = spool.tile([128, NCH], dt.float32)
            nc.scalar.activation(sq[:], prev_st[:], AF.Square,
                                 accum_out=stats[:, 1:2])
            ps_stats = pt.tile([1, 2], dt.float32, tag="tail", bufs=1)
            nc.tensor.matmul(ps_stats[:], ones_col, stats[:], start=True, stop=True)
            mean2 = spool.tile([1, 2], dt.float32)
            nc.vector.tensor_scalar_mul(mean2[:], ps_stats[:], 1.0 / H)
            musd = spool.tile([1, 2], dt.float32)
            nc.vector.tensor_copy(musd[:, 0:1], mean2[:, 0:1])
            mu2 = spool.tile([1, 1], dt.float32)
            nc.vector.tensor_mul(mu2[:], mean2[:, 0:1], mean2[:, 0:1])
            var = spool.tile([1, 1], dt.float32)
            nc.vector.tensor_sub(var[:], mean2[:, 1:2], mu2[:])
            nc.vector.tensor_scalar_add(var[:], var[:], LN_EPS)
            std = spool.tile([1, 1], dt.float32)
            nc.scalar.activation(std[:], var[:], AF.Sqrt)
            nc.vector.reciprocal(musd[:, 1:2], std[:])
            ps_bc = pt.tile([128, 2], dt.float32, tag="tail", bufs=1)
            nc.tensor.matmul(ps_bc[:], ones_row, musd[:], start=True, stop=True)
            bc = spool.tile([128, 2], dt.float32)
            nc.vector.tensor_copy(bc[:], ps_bc[:])
            # y = ((x - mu) * rstd) * gamma + beta
            y = spool.tile([128, NCH], dt.float32)
            nc.vector.tensor_scalar(y[:], prev_st[:], bc[:, 0:1], bc[:, 1:2],
                                    op0=ALU.subtract, op1=ALU.mult)
            nc.vector.tensor_mul(y[:], y[:], gam_sb[:])
            nc.vector.tensor_add(y[:], y[:], bet_sb[:])
            ln_bf = spool.tile([128, NCH], dt.bfloat16)
            nc.vector.tensor_copy(ln_bf[:], y[:])

            # ---- Linear (row shard) + residual ----
            ps_pred = pt.tile([1, S], dt.float32, tag="tail", bufs=1)
            nc.tensor.matmul(ps_pred[:], one1f, lb_sb[:], start=True, stop=False)
            for n in range(NCH):
                nc.tensor.matmul(ps_pred[:], ln_bf[:, n:n + 1], lw_sb[:, n, :],
                                 start=False, stop=(n == NCH - 1))
            pred = spool.tile([1, S], dt.float32)
            nc.vector.tensor_add(pred[:], ps_pred[:], xres_sb[:])
            nc.sync.dma_start(out=out_p.ap(), in_=pred[:])

    nc.compile()
    return nc


def _prep_inputs(inputs):
    """Full inputs -> list of 8 per-core in_maps (host-side shard/transpose)."""
    x = np.asarray(inputs["x"], np.float32)
    hidden = np.asarray(inputs["hidden"], np.float32)
    w_ih = np.asarray(inputs["w_ih"], np.float32)
    w_hh = np.asarray(inputs["w_hh"], np.float32)
    b_ih = np.asarray(inputs["b_ih"], np.float32)
    b_hh = np.asarray(inputs["b_hh"], np.float32)
    ln_gamma = np.asarray(inputs["ln_gamma"], np.float32)
    ln_beta = np.asarray(inputs["ln_beta"], np.float32)
    lin_w = np.asarray(inputs["lin_w"], np.float32)
    lin_b = np.asarray(inputs["lin_b"], np.float32)

    x_t = x[0, -1]                        # [H]
    hid = hidden[:, 0]                    # [L, H]

    def shard_gates(w):
        # [L,3H,H] -> [8, L, NCH, 128, G]  (W^T, contraction-chunked)
        a = w.astype(BF16).reshape(L, 3, NCORES, S, H)
        a = a.transpose(2, 0, 1, 3, 4).reshape(NCORES, L, G, H)
        a = a.transpose(0, 1, 3, 2)                      # [8, L, H, G]
        a = a.reshape(NCORES, L, 128, NCH, G)            # partition-major
        return np.ascontiguousarray(a)                   # [8, L, 128, NCH, G]

    wi8 = shard_gates(w_ih)
    wh8 = shard_gates(w_hh)

    lwt = lin_w.astype(BF16).reshape(NCORES, S, H)        # row shards
    lwt = lwt.transpose(0, 2, 1)                          # [8, H, S]
    lw8 = np.ascontiguousarray(lwt.reshape(NCORES, 128, NCH, S))  # [8,128,NCH,S]

    xst = np.ascontiguousarray(x_t.astype(BF16).reshape(128, NCH))
    hst = np.ascontiguousarray(hid.astype(BF16).reshape(L, 128, NCH).transpose(1, 0, 2))
    gamt = np.ascontiguousarray(ln_gamma.reshape(128, NCH))
    bett = np.ascontiguousarray(ln_beta.reshape(128, NCH))
    onesm = np.ones((128, 128), np.float32)

    bi = b_ih.reshape(L, 3, NCORES, S)
    bh = b_hh.reshape(L, 3, NCORES, S)

    in_maps = []
    for k in range(NCORES):
        brz = (bi[:, 0:2, k] + bh[:, 0:2, k]).reshape(1, L * 2 * S).astype(BF16)
        bin_ = bi[:, 2, k].reshape(1, L * S).astype(BF16)
        bhn = bh[:, 2, k].reshape(1, L * S).astype(BF16)
        in_maps.append({
            "wi": wi8[k],
            "wh": wh8[k],
            "lw": lw8[k],
            "xst": xst,
            "hst": hst,
            "hch": np.ascontiguousarray(hid[:, k * S:(k + 1) * S]).reshape(1, L * S),
            "brz": np.ascontiguousarray(brz),
            "bin": np.ascontiguousarray(bin_),
            "bhn": np.ascontiguousarray(bhn),
            "lb": lin_b[k * S:(k + 1) * S].reshape(1, S).copy(),
            "xres": x_t[k * S:(k + 1) * S].reshape(1, S).copy(),
            "gam": gamt,
            "bet": bett,
            "ones": onesm,
        })
    return in_maps


def _run(inputs, trace=False, tmpdir=None):
    import concourse.bass_utils as bass_utils
    if "nc" not in _cache:
        _cache["nc"] = _build_nc()
    nc = _cache["nc"]
    in_maps = _prep_inputs(inputs)
    res = bass_utils.run_bass_kernel_spmd(
        nc, in_maps, core_ids=list(range(NCORES)), trace=trace, tmpdir=tmpdir)
    pred = np.concatenate([res.results[k]["out_p"][0] for k in range(NCORES)])
    new_hidden = np.stack(
        [np.concatenate([res.results[k]["out_h"][l] for k in range(NCORES)])
         for l in range(L)])[:, None, :]
    return (pred[None, :].astype(np.float32),
            new_hidden.astype(np.float32)), res


def kernel(**inputs):
    out, _ = _run(inputs)
    return out


# revision 9
# speedup vs baseline: 1.0889x; 1.0889x over previous
"""Trainium2 Bass kernel for a 6-layer GRU stack (T=1, B=1, H=2048) + LayerNorm +
Linear with residual, tensor-parallel over 8 NeuronCores.

Contract: kernel(**inputs) takes the FULL unsharded inputs (as produced by the
reference setup_inputs) and returns the full outputs (pred [1,2048],
new_hidden [6,1,2048]).

Strategy (hardcoded for L=6, H=2048, 8 cores):
  - Shard each layer's 3H=6144 gate rows: core k owns rows [k*256,(k+1)*256) of
    each of the r/z/n gate blocks (768 rows/core/matrix).
  - Host-side: slice + transpose weights to W^T, chunk the contraction dim into
    16 chunks of 128 (chunk n holds contraction elements {16p+n}), cast bf16.
  - Device: GEMV = 16 matmuls with a [128,1] slice of the hidden-state
    stationary tile ([128,16] = h.reshape(128,16)) against [128,768] moving
    weight tiles, accumulating in PSUM.  Biases folded in via 1-contraction
    matmuls against a ones stationary.
  - Gate nonlinearities on [1,768]-ish tiles (ScalarE sigmoid/tanh, VectorE TT).
  - Per layer: AllGather of the 256-element h_new chunk (bf16) via DRAM bounce
    to rebuild the full 2048 hidden state on every core.
  - Tail: LayerNorm computed redundantly on every core from the final gathered
    h (partition-sum / broadcast via ones-matmuls), then a row-sharded Linear
    (256 out rows/core) + residual; host concatenates the 8 chunks.
"""
import numpy as np
import ml_dtypes

L = 6
H = 2048
NCORES = 8
S = H // NCORES          # 256  per-core hidden chunk
G = 3 * S                # 768  gate rows per core
NCH = H // 128           # 16   contraction chunks
LN_EPS = 1e-5

BF16 = ml_dtypes.bfloat16

_cache = {}


def _build_nc():
    import concourse.bacc as bacc
    import concourse.mybir as mybir
    import concourse.tile as tile

    dt = mybir.dt
    AF = mybir.ActivationFunctionType
    ALU = mybir.AluOpType

    nc = bacc.Bacc("TRN2", target_bir_lowering=False, debug=False,
                   num_devices=NCORES)

    wi = nc.dram_tensor("wi", [L, 128, NCH, G], dt.bfloat16, kind="ExternalInput")
    wh = nc.dram_tensor("wh", [L, 128, NCH, G], dt.bfloat16, kind="ExternalInput")
    lw = nc.dram_tensor("lw", [128, NCH, S], dt.bfloat16, kind="ExternalInput")
    xst = nc.dram_tensor("xst", [128, NCH], dt.bfloat16, kind="ExternalInput")
    hst = nc.dram_tensor("hst", [128, L, NCH], dt.bfloat16, kind="ExternalInput")
    hch = nc.dram_tensor("hch", [1, L * S], dt.float32, kind="ExternalInput")
    brz = nc.dram_tensor("brz", [1, L * 2 * S], dt.bfloat16, kind="ExternalInput")
    bin_ = nc.dram_tensor("bin", [1, L * S], dt.bfloat16, kind="ExternalInput")
    bhn = nc.dram_tensor("bhn", [1, L * S], dt.bfloat16, kind="ExternalInput")
    lb = nc.dram_tensor("lb", [1, S], dt.float32, kind="ExternalInput")
    xres = nc.dram_tensor("xres", [1, S], dt.float32, kind="ExternalInput")
    gam = nc.dram_tensor("gam", [128, NCH], dt.float32, kind="ExternalInput")
    bet = nc.dram_tensor("bet", [128, NCH], dt.float32, kind="ExternalInput")
    ones = nc.dram_tensor("ones", [128, 128], dt.float32, kind="ExternalInput")

    out_h = nc.dram_tensor("out_h", [L, S], dt.float32, kind="ExternalOutput")
    out_p = nc.dram_tensor("out_p", [1, S], dt.float32, kind="ExternalOutput")

    RG = [list(range(NCORES))]

    with tile.TileContext(nc) as tc:
        with tc.tile_pool(name="wpool", bufs=2) as wpool, \
             tc.tile_pool(name="spool", bufs=2) as spool, \
             tc.tile_pool(name="cpool", bufs=1) as cpool, \
             tc.tile_pool(name="psum", bufs=2, space="PSUM") as pp, \
             tc.tile_pool(name="ptail", bufs=2, space="PSUM") as pt, \
             tc.tile_pool(name="dram", bufs=2, space="DRAM") as dpool:

            # ---- constants / small inputs (loaded once) ----
            xst_sb = cpool.tile([128, NCH], dt.bfloat16)
            nc.sync.dma_start(out=xst_sb[:], in_=xst.ap())
            hst_sb = cpool.tile([128, L, NCH], dt.bfloat16)
            nc.sync.dma_start(out=hst_sb[:], in_=hst.ap())
            hch_sb = cpool.tile([1, L * S], dt.float32)
            nc.sync.dma_start(out=hch_sb[:], in_=hch.ap())
            brz_sb = cpool.tile([1, L * 2 * S], dt.bfloat16)
            nc.sync.dma_start(out=brz_sb[:], in_=brz.ap())
            bin_sb = cpool.tile([1, L * S], dt.bfloat16)
            nc.sync.dma_start(out=bin_sb[:], in_=bin_.ap())
            bhn_sb = cpool.tile([1, L * S], dt.bfloat16)
            nc.sync.dma_start(out=bhn_sb[:], in_=bhn.ap())
            lb_sb = cpool.tile([1, S], dt.float32)
            nc.sync.dma_start(out=lb_sb[:], in_=lb.ap())
            xres_sb = cpool.tile([1, S], dt.float32)
            nc.sync.dma_start(out=xres_sb[:], in_=xres.ap())
            gam_sb = cpool.tile([128, NCH], dt.float32)
            nc.sync.dma_start(out=gam_sb[:], in_=gam.ap())
            bet_sb = cpool.tile([128, NCH], dt.float32)
            nc.sync.dma_start(out=bet_sb[:], in_=bet.ap())
            ones_sb = cpool.tile([128, 128], dt.float32)
            nc.sync.dma_start(out=ones_sb[:], in_=ones.ap())
            onesb_sb = cpool.tile([1, 16], dt.bfloat16)
            nc.gpsimd.dma_start(out=onesb_sb[:], in_=ones.ap()[0:1, 0:16])
            lw_sb = cpool.tile([128, NCH, S], dt.bfloat16)
            nc.sync.dma_start(out=lw_sb[:], in_=lw.ap())

            one1 = onesb_sb[0:1, 0:1]         # [1,1] bf16 bias-matmul stationary
            one1f = ones_sb[0:1, 0:1]         # [1,1] fp32
            ones_col = ones_sb[:, 0:1]        # [128,1] partition-sum stationary
            ones_row = ones_sb[0:1, :]        # [1,128] broadcast stationary

            prev_st = xst_sb                  # layer-0 gi stationary = x_t

            psA = {}   # [1,512] psum: r|z preactivation (gi+gh+biases)
            psB = {}   # [1,512] psum: gi_n (cols 0:S) | gh_n (cols S:2S)
            wis = {}
            whs = {}

            # warmup collective: absorbs ncfw first-call cost + aligns cores
            # while the first weight DMAs stream.
            wu_in = dpool.tile([1, 8], dt.float32, tag="wuin")
            nc.sync.dma_start(out=wu_in[:], in_=ones.ap()[0:1, 0:8])
            wu_out = dpool.tile([NCORES, 8], dt.float32, tag="wuout",
                                addr_space="Shared")
            nc.gpsimd.collective_compute(
                "AllGather", ALU.bypass, replica_groups=RG,
                ins=[wu_in[:]], outs=[wu_out[:]],
            )

            def emit_wdma(l):
                # wi on the ACT HWDGE ring, wh on the GPSIMD SWDGE ring:
                # two independent issue streams keep the SDMA engines fed.
                wi_sb = wpool.tile([128, NCH, G], dt.bfloat16, tag="wi", bufs=3)
                nc.scalar.dma_start(out=wi_sb[:], in_=wi.ap()[l])
                wis[l] = wi_sb
                wh_sb = wpool.tile([128, NCH, G], dt.bfloat16, tag="wh", bufs=3)
                nc.gpsimd.dma_start(out=wh_sb[:], in_=wh.ap()[l])
                whs[l] = wh_sb

            def emit_wgh(l):
                wh_sb = whs[l]
                A = pp.tile([1, 2 * S], dt.float32, tag="A", bufs=3)
                B = pp.tile([1, 2 * S], dt.float32, tag="B", bufs=3)
                psA[l], psB[l] = A, B
                nc.tensor.matmul(A[:], one1,
                                 brz_sb[:, l * 2 * S:(l + 1) * 2 * S],
                                 start=True, stop=False)
                nc.tensor.matmul(B[:, 0:S], one1, bin_sb[:, l * S:(l + 1) * S],
                                 start=True, stop=False)
                nc.tensor.matmul(B[:, S:2 * S], one1, bhn_sb[:, l * S:(l + 1) * S],
                                 start=True, stop=False)
                for n in range(NCH):
                    st = hst_sb[:, l, n:n + 1]
                    nc.tensor.matmul(A[:], st, wh_sb[:, n, 0:2 * S],
                                     start=False, stop=False)
                    nc.tensor.matmul(B[:, S:2 * S], st, wh_sb[:, n, 2 * S:G],
                                     start=False, stop=(n == NCH - 1))

            def emit_gi(l):
                A, B = psA[l], psB[l]
                wi_sb = wis[l]
                for n in range(NCH):       # A first: sigmoid starts earlier
                    st = prev_st[:, n:n + 1]
                    nc.tensor.matmul(A[:], st, wi_sb[:, n, 0:2 * S],
                                     start=False, stop=(n == NCH - 1))
                for n in range(NCH):
                    st = prev_st[:, n:n + 1]
                    nc.tensor.matmul(B[:, 0:S], st, wi_sb[:, n, 2 * S:G],
                                     start=False, stop=(n == NCH - 1))

            emit_wdma(0)
            emit_wdma(1)
            emit_wdma(2)
            # PE warmup: keep HAM busy while the first weights stream in
            ps_warm = pp.tile([1, NCH], dt.float32, tag="warm", bufs=1)
            for i in range(24):
                nc.tensor.matmul(ps_warm[:], xst_sb[:, 0:1], xst_sb[:, :],
                                 start=(i == 0), stop=(i == 23))
            emit_wgh(0)
            for l in range(L):
                if l + 3 < L:
                    emit_wdma(l + 3)
                if l + 1 < L:
                    emit_wgh(l + 1)
                emit_gi(l)
                A, B = psA[l], psB[l]

                # ---- gates ----
                sig = spool.tile([1, 2 * S], dt.float32, tag="sig")
                nc.scalar.activation(sig[:], A[:], AF.Sigmoid)
                t_ = spool.tile([1, S], dt.float32, tag="t")
                nc.vector.tensor_mul(t_[:], sig[:, 0:S], B[:, S:2 * S])
                u_ = spool.tile([1, S], dt.float32, tag="u")
                nc.vector.tensor_add(u_[:], B[:, 0:S], t_[:])
                nn_ = spool.tile([1, S], dt.float32, tag="nn")
                nc.scalar.activation(nn_[:], u_[:], AF.Tanh)
                d_ = spool.tile([1, S], dt.float32, tag="d")
                nc.vector.tensor_sub(d_[:], hch_sb[:, l * S:(l + 1) * S], nn_[:])
                zd = spool.tile([1, S], dt.float32, tag="zd")
                nc.vector.tensor_mul(zd[:], sig[:, S:2 * S], d_[:])
                hnew = spool.tile([1, S], dt.bfloat16, tag="hnew")
                nc.vector.tensor_add(hnew[:], nn_[:], zd[:])

                # fp32 hidden-state output (off critical path, SWDGE cast)
                nc.gpsimd.dma_start(out=out_h.ap()[l:l + 1, :], in_=hnew[:])

                # ---- exchange: AllGather, reload as stationary ----
                ag_in = dpool.tile([1, S], dt.bfloat16, tag="agin")
                nc.sync.dma_start(out=ag_in[:], in_=hnew[:])
                ag_out = dpool.tile([128, NCH], dt.bfloat16, tag="agout",
                                    addr_space="Shared")
                nc.gpsimd.collective_compute(
                    "AllGather", ALU.bypass, replica_groups=RG,
                    ins=[ag_in[:]], outs=[ag_out[:]],
                )
                h_st = spool.tile([128, NCH], dt.bfloat16, tag="hstat")
                nc.sync.dma_start(out=h_st[:], in_=ag_out[:])
                prev_st = h_st

            # ---- LayerNorm on the gathered top hidden state ----
            stats = spool.tile([128, 2], dt.float32)
            nc.vector.reduce_sum(stats[:, 0:1], prev_st[:], axis=mybir.AxisListType.X)
            sq = spool.tile([128, NCH], dt.float32)
            nc.scalar.activation(sq[:], prev_st[:], AF.Square,
                                 accum_out=stats[:, 1:2])
            ps_stats = pt.tile([1, 2], dt.float32, tag="tail", bufs=1)
            nc.tensor.matmul(ps_stats[:], ones_col, stats[:], start=True, stop=True)
            mean2 = spool.tile([1, 2], dt.float32)
            nc.vector.tensor_scalar_mul(mean2[:], ps_stats[:], 1.0 / H)
            musd = spool.tile([1, 2], dt.float32)
            nc.vector.tensor_copy(musd[:, 0:1], mean2[:, 0:1])
            mu2 = spool.tile([1, 1], dt.float32)
            nc.vector.tensor_mul(mu2[:], mean2[:, 0:1], mean2[:, 0:1])
            var = spool.tile([1, 1], dt.float32)
            nc.vector.tensor_sub(var[:], mean2[:, 1:2], mu2[:])
            nc.vector.tensor_scalar_add(var[:], var[:], LN_EPS)
            std = spool.tile([1, 1], dt.float32)
            nc.scalar.activation(std[:], var[:], AF.Sqrt)
            nc.vector.reciprocal(musd[:, 1:2], std[:])
            ps_bc = pt.tile([128, 2], dt.float32, tag="tail", bufs=1)
            nc.tensor.matmul(ps_bc[:], ones_row, musd[:], start=True, stop=True)
            bc = spool.tile([128, 2], dt.float32)
            nc.vector.tensor_copy(bc[:], ps_bc[:])
            # y = ((x - mu) * rstd) * gamma + beta
            y = spool.tile([128, NCH], dt.float32)
            nc.vector.tensor_scalar(y[:], prev_st[:], bc[:, 0:1], bc[:, 1:2],
                                    op0=ALU.subtract, op1=ALU.mult)
            nc.vector.tensor_mul(y[:], y[:], gam_sb[:])
            nc.vector.tensor_add(y[:], y[:], bet_sb[:])
            ln_bf = spool.tile([128, NCH], dt.bfloat16)
            nc.vector.tensor_copy(ln_bf[:], y[:])

            # ---- Linear (row shard) + residual ----
            ps_pred = pt.tile([1, S], dt.float32, tag="tail", bufs=1)
            nc.tensor.matmul(ps_pred[:], one1f, lb_sb[:], start=True, stop=False)
            for n in range(NCH):
                nc.tensor.matmul(ps_pred[:], ln_bf[:, n:n + 1], lw_sb[:, n, :],
                                 start=False, stop=(n == NCH - 1))
            pred = spool.tile([1, S], dt.float32)
            nc.vector.tensor_add(pred[:], ps_pred[:], xres_sb[:])
            nc.sync.dma_start(out=out_p.ap(), in_=pred[:])

    nc.compile()
    return nc


def _prep_inputs(inputs):
    """Full inputs -> list of 8 per-core in_maps (host-side shard/transpose)."""
    x = np.asarray(inputs["x"], np.float32)
    hidden = np.asarray(inputs["hidden"], np.float32)
    w_ih = np.asarray(inputs["w_ih"], np.float32)
    w_hh = np.asarray(inputs["w_hh"], np.float32)
    b_ih = np.asarray(inputs["b_ih"], np.float32)
    b_hh = np.asarray(inputs["b_hh"], np.float32)
    ln_gamma = np.asarray(inputs["ln_gamma"], np.float32)
    ln_beta = np.asarray(inputs["ln_beta"], np.float32)
    lin_w = np.asarray(inputs["lin_w"], np.float32)
    lin_b = np.asarray(inputs["lin_b"], np.float32)

    x_t = x[0, -1]                        # [H]
    hid = hidden[:, 0]                    # [L, H]

    def shard_gates(w):
        # [L,3H,H] -> [8, L, NCH, 128, G]  (W^T, contraction-chunked)
        a = w.astype(BF16).reshape(L, 3, NCORES, S, H)
        a = a.transpose(2, 0, 1, 3, 4).reshape(NCORES, L, G, H)
        a = a.transpose(0, 1, 3, 2)                      # [8, L, H, G]
        a = a.reshape(NCORES, L, 128, NCH, G)            # partition-major
        return np.ascontiguousarray(a)                   # [8, L, 128, NCH, G]

    wi8 = shard_gates(w_ih)
    wh8 = shard_gates(w_hh)

    lwt = lin_w.astype(BF16).reshape(NCORES, S, H)        # row shards
    lwt = lwt.transpose(0, 2, 1)                          # [8, H, S]
    lw8 = np.ascontiguousarray(lwt.reshape(NCORES, 128, NCH, S))  # [8,128,NCH,S]

    xst = np.ascontiguousarray(x_t.astype(BF16).reshape(128, NCH))
    hst = np.ascontiguousarray(hid.astype(BF16).reshape(L, 128, NCH).transpose(1, 0, 2))
    gamt = np.ascontiguousarray(ln_gamma.reshape(128, NCH))
    bett = np.ascontiguousarray(ln_beta.reshape(128, NCH))
    onesm = np.ones((128, 128), np.float32)

    bi = b_ih.reshape(L, 3, NCORES, S)
    bh = b_hh.reshape(L, 3, NCORES, S)

    in_maps = []
    for k in range(NCORES):
        brz = (bi[:, 0:2, k] + bh[:, 0:2, k]).reshape(1, L * 2 * S).astype(BF16)
        bin_ = bi[:, 2, k].reshape(1, L * S).astype(BF16)
        bhn = bh[:, 2, k].reshape(1, L * S).astype(BF16)
        in_maps.append({
            "wi": wi8[k],
            "wh": wh8[k],
            "lw": lw8[k],
            "xst": xst,
            "hst": hst,
            "hch": np.ascontiguousarray(hid[:, k * S:(k + 1) * S]).reshape(1, L * S),
            "brz": np.ascontiguousarray(brz),
            "bin": np.ascontiguousarray(bin_),
            "bhn": np.ascontiguousarray(bhn),
            "lb": lin_b[k * S:(k + 1) * S].reshape(1, S).copy(),
            "xres": x_t[k * S:(k + 1) * S].reshape(1, S).copy(),
            "gam": gamt,
            "bet": bett,
            "ones": onesm,
        })
    return in_maps


def _run(inputs, trace=False, tmpdir=None):
    import concourse.bass_utils as bass_utils
    if "nc" not in _cache:
        _cache["nc"] = _build_nc()
    nc = _cache["nc"]
    in_maps = _prep_inputs(inputs)
    res = bass_utils.run_bass_kernel_spmd(
        nc, in_maps, core_ids=list(range(NCORES)), trace=trace, tmpdir=tmpdir)
    pred = np.concatenate([res.results[k]["out_p"][0] for k in range(NCORES)])
    new_hidden = np.stack(
        [np.concatenate([res.results[k]["out_h"][l] for k in range(NCORES)])
         for l in range(L)])[:, None, :]
    return (pred[None, :].astype(np.float32),
            new_hidden.astype(np.float32)), res


def kernel(**inputs):
    out, _ = _run(inputs)
    return out


# revision 10
# speedup vs baseline: 1.1004x; 1.0106x over previous
"""Trainium2 Bass kernel for a 6-layer GRU stack (T=1, B=1, H=2048) + LayerNorm +
Linear with residual, tensor-parallel over 8 NeuronCores.

Contract: kernel(**inputs) takes the FULL unsharded inputs (as produced by the
reference setup_inputs) and returns the full outputs (pred [1,2048],
new_hidden [6,1,2048]).

Strategy (hardcoded for L=6, H=2048, 8 cores):
  - Shard each layer's 3H=6144 gate rows: core k owns rows [k*256,(k+1)*256) of
    each of the r/z/n gate blocks (768 rows/core/matrix).
  - Host-side: slice + transpose weights to W^T, chunk the contraction dim into
    16 chunks of 128 (chunk n holds contraction elements {16p+n}), cast bf16.
  - Device: GEMV = 16 matmuls with a [128,1] slice of the hidden-state
    stationary tile ([128,16] = h.reshape(128,16)) against [128,768] moving
    weight tiles, accumulating in PSUM.  Biases folded in via 1-contraction
    matmuls against a ones stationary.
  - Gate nonlinearities on [1,768]-ish tiles (ScalarE sigmoid/tanh, VectorE TT).
  - Per layer: AllGather of the 256-element h_new chunk (bf16) via DRAM bounce
    to rebuild the full 2048 hidden state on every core.
  - Tail: LayerNorm computed redundantly on every core from the final gathered
    h (partition-sum / broadcast via ones-matmuls), then a row-sharded Linear
    (256 out rows/core) + residual; host concatenates the 8 chunks.
"""
import numpy as np
import ml_dtypes

L = 6
H = 2048
NCORES = 8
S = H // NCORES          # 256  per-core hidden chunk
G = 3 * S                # 768  gate rows per core
NCH = H // 128           # 16   contraction chunks
LN_EPS = 1e-5

BF16 = ml_dtypes.bfloat16

_cache = {}


def _build_nc():
    import concourse.bacc as bacc
    import concourse.mybir as mybir
    import concourse.tile as tile

    dt = mybir.dt
    AF = mybir.ActivationFunctionType
    ALU = mybir.AluOpType

    nc = bacc.Bacc("TRN2", target_bir_lowering=False, debug=False,
                   num_devices=NCORES)

    wi = nc.dram_tensor("wi", [L, 128, NCH, G], dt.bfloat16, kind="ExternalInput")
    wh = nc.dram_tensor("wh", [L, 128, NCH, G], dt.bfloat16, kind="ExternalInput")
    lw = nc.dram_tensor("lw", [128, NCH, S], dt.bfloat16, kind="ExternalInput")
    xst = nc.dram_tensor("xst", [128, NCH], dt.bfloat16, kind="ExternalInput")
    hst = nc.dram_tensor("hst", [128, L, NCH], dt.bfloat16, kind="ExternalInput")
    hch = nc.dram_tensor("hch", [1, L * S], dt.float32, kind="ExternalInput")
    brz = nc.dram_tensor("brz", [1, L * 2 * S], dt.bfloat16, kind="ExternalInput")
    bin_ = nc.dram_tensor("bin", [1, L * S], dt.bfloat16, kind="ExternalInput")
    bhn = nc.dram_tensor("bhn", [1, L * S], dt.bfloat16, kind="ExternalInput")
    lb = nc.dram_tensor("lb", [1, S], dt.float32, kind="ExternalInput")
    xres = nc.dram_tensor("xres", [1, S], dt.float32, kind="ExternalInput")
    gam = nc.dram_tensor("gam", [128, NCH], dt.float32, kind="ExternalInput")
    bet = nc.dram_tensor("bet", [128, NCH], dt.float32, kind="ExternalInput")
    ones = nc.dram_tensor("ones", [128, 128], dt.float32, kind="ExternalInput")

    out_h = nc.dram_tensor("out_h", [L, S], dt.float32, kind="ExternalOutput")
    out_p = nc.dram_tensor("out_p", [1, S], dt.float32, kind="ExternalOutput")

    RG = [list(range(NCORES))]

    with tile.TileContext(nc) as tc:
        with tc.tile_pool(name="wpool", bufs=2) as wpool, \
             tc.tile_pool(name="spool", bufs=2) as spool, \
             tc.tile_pool(name="cpool", bufs=1) as cpool, \
             tc.tile_pool(name="psum", bufs=2, space="PSUM") as pp, \
             tc.tile_pool(name="ptail", bufs=2, space="PSUM") as pt, \
             tc.tile_pool(name="dram", bufs=2, space="DRAM") as dpool:

            # ---- constants / small inputs (loaded once) ----
            xst_sb = cpool.tile([128, NCH], dt.bfloat16)
            nc.sync.dma_start(out=xst_sb[:], in_=xst.ap())
            hst_sb = cpool.tile([128, L, NCH], dt.bfloat16)
            nc.sync.dma_start(out=hst_sb[:], in_=hst.ap())
            hch_sb = cpool.tile([1, L * S], dt.float32)
            nc.sync.dma_start(out=hch_sb[:], in_=hch.ap())
            brz_sb = cpool.tile([1, L * 2 * S], dt.bfloat16)
            nc.sync.dma_start(out=brz_sb[:], in_=brz.ap())
            bin_sb = cpool.tile([1, L * S], dt.bfloat16)
            nc.sync.dma_start(out=bin_sb[:], in_=bin_.ap())
            bhn_sb = cpool.tile([1, L * S], dt.bfloat16)
            nc.sync.dma_start(out=bhn_sb[:], in_=bhn.ap())
            lb_sb = cpool.tile([1, S], dt.float32)
            nc.sync.dma_start(out=lb_sb[:], in_=lb.ap())
            xres_sb = cpool.tile([1, S], dt.float32)
            nc.sync.dma_start(out=xres_sb[:], in_=xres.ap())
            gam_sb = cpool.tile([128, NCH], dt.float32)
            nc.sync.dma_start(out=gam_sb[:], in_=gam.ap())
            bet_sb = cpool.tile([128, NCH], dt.float32)
            nc.sync.dma_start(out=bet_sb[:], in_=bet.ap())
            ones_sb = cpool.tile([128, 128], dt.float32)
            nc.sync.dma_start(out=ones_sb[:], in_=ones.ap())
            onesb_sb = cpool.tile([1, 16], dt.bfloat16)
            nc.gpsimd.dma_start(out=onesb_sb[:], in_=ones.ap()[0:1, 0:16])
            lw_sb = cpool.tile([128, NCH, S], dt.bfloat16)
            nc.sync.dma_start(out=lw_sb[:], in_=lw.ap())

            one1 = onesb_sb[0:1, 0:1]         # [1,1] bf16 bias-matmul stationary
            one1f = ones_sb[0:1, 0:1]         # [1,1] fp32
            ones_col = ones_sb[:, 0:1]        # [128,1] partition-sum stationary
            ones_row = ones_sb[0:1, :]        # [1,128] broadcast stationary

            prev_st = xst_sb                  # layer-0 gi stationary = x_t

            psA = {}   # [1,512] psum: r|z preactivation (gi+gh+biases)
            psB = {}   # [1,512] psum: gi_n (cols 0:S) | gh_n (cols S:2S)
            wis = {}
            whs = {}

            # warmup collective: absorbs ncfw first-call cost + aligns cores
            # while the first weight DMAs stream.
            wu_in = dpool.tile([1, 8], dt.float32, tag="wuin")
            nc.sync.dma_start(out=wu_in[:], in_=ones.ap()[0:1, 0:8])
            wu_out = dpool.tile([NCORES, 8], dt.float32, tag="wuout",
                                addr_space="Shared")
            nc.gpsimd.collective_compute(
                "AllGather", ALU.bypass, replica_groups=RG,
                ins=[wu_in[:]], outs=[wu_out[:]],
            )

            def emit_wdma(l):
                # wi on the ACT HWDGE ring, wh on the GPSIMD SWDGE ring:
                # two independent issue streams keep the SDMA engines fed.
                wi_sb = wpool.tile([128, NCH, G], dt.bfloat16, tag="wi", bufs=3)
                nc.gpsimd.dma_start(out=wi_sb[:], in_=wi.ap()[l])
                wis[l] = wi_sb
                wh_sb = wpool.tile([128, NCH, G], dt.bfloat16, tag="wh", bufs=3)
                nc.gpsimd.dma_start(out=wh_sb[:], in_=wh.ap()[l])
                whs[l] = wh_sb

            def emit_wgh(l):
                wh_sb = whs[l]
                A = pp.tile([1, 2 * S], dt.float32, tag="A", bufs=3)
                B = pp.tile([1, 2 * S], dt.float32, tag="B", bufs=3)
                psA[l], psB[l] = A, B
                nc.tensor.matmul(A[:], one1,
                                 brz_sb[:, l * 2 * S:(l + 1) * 2 * S],
                                 start=True, stop=False)
                nc.tensor.matmul(B[:, 0:S], one1, bin_sb[:, l * S:(l + 1) * S],
                                 start=True, stop=False)
                nc.tensor.matmul(B[:, S:2 * S], one1, bhn_sb[:, l * S:(l + 1) * S],
                                 start=True, stop=False)
                for n in range(NCH):
                    st = hst_sb[:, l, n:n + 1]
                    nc.tensor.matmul(A[:], st, wh_sb[:, n, 0:2 * S],
                                     start=False, stop=False)
                    nc.tensor.matmul(B[:, S:2 * S], st, wh_sb[:, n, 2 * S:G],
                                     start=False, stop=(n == NCH - 1))

            def emit_gi(l):
                A, B = psA[l], psB[l]
                wi_sb = wis[l]
                for n in range(NCH):       # A first: sigmoid starts earlier
                    st = prev_st[:, n:n + 1]
                    nc.tensor.matmul(A[:], st, wi_sb[:, n, 0:2 * S],
                                     start=False, stop=(n == NCH - 1))
                for n in range(NCH):
                    st = prev_st[:, n:n + 1]
                    nc.tensor.matmul(B[:, 0:S], st, wi_sb[:, n, 2 * S:G],
                                     start=False, stop=(n == NCH - 1))

            emit_wdma(0)
            emit_wdma(1)
            emit_wdma(2)
            # PE warmup: keep HAM busy while the first weights stream in
            ps_warm = pp.tile([1, NCH], dt.float32, tag="warm", bufs=1)
            for i in range(24):
                nc.tensor.matmul(ps_warm[:], xst_sb[:, 0:1], xst_sb[:, :],
                                 start=(i == 0), stop=(i == 23))
            emit_wgh(0)
            for l in range(L):
                if l + 3 < L:
                    emit_wdma(l + 3)
                if l + 1 < L:
                    emit_wgh(l + 1)
                emit_gi(l)
                A, B = psA[l], psB[l]

                # ---- gates ----
                sig = spool.tile([1, 2 * S], dt.float32, tag="sig")
                nc.scalar.activation(sig[:], A[:], AF.Sigmoid)
                t_ = spool.tile([1, S], dt.float32, tag="t")
                nc.vector.tensor_mul(t_[:], sig[:, 0:S], B[:, S:2 * S])
                u_ = spool.tile([1, S], dt.float32, tag="u")
                nc.vector.tensor_add(u_[:], B[:, 0:S], t_[:])
                nn_ = spool.tile([1, S], dt.float32, tag="nn")
                nc.scalar.activation(nn_[:], u_[:], AF.Tanh)
                d_ = spool.tile([1, S], dt.float32, tag="d")
                nc.vector.tensor_sub(d_[:], hch_sb[:, l * S:(l + 1) * S], nn_[:])
                zd = spool.tile([1, S], dt.float32, tag="zd")
                nc.vector.tensor_mul(zd[:], sig[:, S:2 * S], d_[:])
                hnew = spool.tile([1, S], dt.bfloat16, tag="hnew")
                nc.vector.tensor_add(hnew[:], nn_[:], zd[:])

                # fp32 hidden-state output (off critical path, SWDGE cast)
                nc.gpsimd.dma_start(out=out_h.ap()[l:l + 1, :], in_=hnew[:])

                # ---- exchange: AllGather, reload as stationary ----
                ag_in = dpool.tile([1, S], dt.bfloat16, tag="agin")
                nc.sync.dma_start(out=ag_in[:], in_=hnew[:])
                ag_out = dpool.tile([128, NCH], dt.bfloat16, tag="agout",
                                    addr_space="Shared")
                nc.gpsimd.collective_compute(
                    "AllGather", ALU.bypass, replica_groups=RG,
                    ins=[ag_in[:]], outs=[ag_out[:]],
                )
                h_st = spool.tile([128, NCH], dt.bfloat16, tag="hstat")
                nc.sync.dma_start(out=h_st[:], in_=ag_out[:])
                prev_st = h_st

            # ---- LayerNorm on the gathered top hidden state ----
            stats = spool.tile([128, 2], dt.float32)
            nc.vector.reduce_sum(stats[:, 0:1], prev_st[:], axis=mybir.AxisListType.X)
            sq = spool.tile([128, NCH], dt.float32)
            nc.scalar.activation(sq[:], prev_st[:], AF.Square,
                                 accum_out=stats[:, 1:2])
            ps_stats = pt.tile([1, 2], dt.float32, tag="tail", bufs=1)
            nc.tensor.matmul(ps_stats[:], ones_col, stats[:], start=True, stop=True)
            mean2 = spool.tile([1, 2], dt.float32)
            nc.vector.tensor_scalar_mul(mean2[:], ps_stats[:], 1.0 / H)
            musd = spool.tile([1, 2], dt.float32)
            nc.vector.tensor_copy(musd[:, 0:1], mean2[:, 0:1])
            mu2 = spool.tile([1, 1], dt.float32)
            nc.vector.tensor_mul(mu2[:], mean2[:, 0:1], mean2[:, 0:1])
            var = spool.tile([1, 1], dt.float32)
            nc.vector.tensor_sub(var[:], mean2[:, 1:2], mu2[:])
            nc.vector.tensor_scalar_add(var[:], var[:], LN_EPS)
            std = spool.tile([1, 1], dt.float32)
            nc.scalar.activation(std[:], var[:], AF.Sqrt)
            nc.vector.reciprocal(musd[:, 1:2], std[:])
            ps_bc = pt.tile([128, 2], dt.float32, tag="tail", bufs=1)
            nc.tensor.matmul(ps_bc[:], ones_row, musd[:], start=True, stop=True)
            bc = spool.tile([128, 2], dt.float32)
            nc.vector.tensor_copy(bc[:], ps_bc[:])
            # y = ((x - mu) * rstd) * gamma + beta
            y = spool.tile([128, NCH], dt.float32)
            nc.vector.tensor_scalar(y[:], prev_st[:], bc[:, 0:1], bc[:, 1:2],
                                    op0=ALU.subtract, op1=ALU.mult)
            nc.vector.tensor_mul(y[:], y[:], gam_sb[:])
            nc.vector.tensor_add(y[:], y[:], bet_sb[:])
            ln_bf = spool.tile([128, NCH], dt.bfloat16)
            nc.vector.tensor_copy(ln_bf[:], y[:])

            # ---- Linear (row shard) + residual ----
            ps_pred = pt.tile([1, S], dt.float32, tag="tail", bufs=1)
            nc.tensor.matmul(ps_pred[:], one1f, lb_sb[:], start=True, stop=False)
            for n in range(NCH):
                nc.tensor.matmul(ps_pred[:], ln_bf[:, n:n + 1], lw_sb[:, n, :],
                                 start=False, stop=(n == NCH - 1))
            pred = spool.tile([1, S], dt.float32)
            nc.vector.tensor_add(pred[:], ps_pred[:], xres_sb[:])
            nc.sync.dma_start(out=out_p.ap(), in_=pred[:])

    nc.compile()
    return nc


def _prep_inputs(inputs):
    """Full inputs -> list of 8 per-core in_maps (host-side shard/transpose)."""
    x = np.asarray(inputs["x"], np.float32)
    hidden = np.asarray(inputs["hidden"], np.float32)
    w_ih = np.asarray(inputs["w_ih"], np.float32)
    w_hh = np.asarray(inputs["w_hh"], np.float32)
    b_ih = np.asarray(inputs["b_ih"], np.float32)
    b_hh = np.asarray(inputs["b_hh"], np.float32)
    ln_gamma = np.asarray(inputs["ln_gamma"], np.float32)
    ln_beta = np.asarray(inputs["ln_beta"], np.float32)
    lin_w = np.asarray(inputs["lin_w"], np.float32)
    lin_b = np.asarray(inputs["lin_b"], np.float32)

    x_t = x[0, -1]                        # [H]
    hid = hidden[:, 0]                    # [L, H]

    def shard_gates(w):
        # [L,3H,H] -> [8, L, NCH, 128, G]  (W^T, contraction-chunked)
        a = w.astype(BF16).reshape(L, 3, NCORES, S, H)
        a = a.transpose(2, 0, 1, 3, 4).reshape(NCORES, L, G, H)
        a = a.transpose(0, 1, 3, 2)                      # [8, L, H, G]
        a = a.reshape(NCORES, L, 128, NCH, G)            # partition-major
        return np.ascontiguousarray(a)                   # [8, L, 128, NCH, G]

    wi8 = shard_gates(w_ih)
    wh8 = shard_gates(w_hh)

    lwt = lin_w.astype(BF16).reshape(NCORES, S, H)        # row shards
    lwt = lwt.transpose(0, 2, 1)                          # [8, H, S]
    lw8 = np.ascontiguousarray(lwt.reshape(NCORES, 128, NCH, S))  # [8,128,NCH,S]

    xst = np.ascontiguousarray(x_t.astype(BF16).reshape(128, NCH))
    hst = np.ascontiguousarray(hid.astype(BF16).reshape(L, 128, NCH).transpose(1, 0, 2))
    gamt = np.ascontiguousarray(ln_gamma.reshape(128, NCH))
    bett = np.ascontiguousarray(ln_beta.reshape(128, NCH))
    onesm = np.ones((128, 128), np.float32)

    bi = b_ih.reshape(L, 3, NCORES, S)
    bh = b_hh.reshape(L, 3, NCORES, S)

    in_maps = []
    for k in range(NCORES):
        brz = (bi[:, 0:2, k] + bh[:, 0:2, k]).reshape(1, L * 2 * S).astype(BF16)
        bin_ = bi[:, 2, k].reshape(1, L * S).astype(BF16)
        bhn = bh[:, 2, k].reshape(1, L * S).astype(BF16)
        in_maps.append({
            "wi": wi8[k],
            "wh": wh8[k],
            "lw": lw8[k],
            "xst": xst,
            "hst": hst,
            "hch": np.ascontiguousarray(hid[:, k * S:(k + 1) * S]).reshape(1, L * S),
            "brz": np.ascontiguousarray(brz),
            "bin": np.ascontiguousarray(bin_),
            "bhn": np.ascontiguousarray(bhn),
            "lb": lin_b[k * S:(k + 1) * S].reshape(1, S).copy(),
            "xres": x_t[k * S:(k + 1) * S].reshape(1, S).copy(),
            "gam": gamt,
            "bet": bett,
            "ones": onesm,
        })
    return in_maps


def _run(inputs, trace=False, tmpdir=None):
    import concourse.bass_utils as bass_utils
    if "nc" not in _cache:
        _cache["nc"] = _build_nc()
    nc = _cache["nc"]
    in_maps = _prep_inputs(inputs)
    res = bass_utils.run_bass_kernel_spmd(
        nc, in_maps, core_ids=list(range(NCORES)), trace=trace, tmpdir=tmpdir)
    pred = np.concatenate([res.results[k]["out_p"][0] for k in range(NCORES)])
    new_hidden = np.stack(
        [np.concatenate([res.results[k]["out_h"][l] for k in range(NCORES)])
         for l in range(L)])[:, None, :]
    return (pred[None, :].astype(np.float32),
            new_hidden.astype(np.float32)), res


def kernel(**inputs):
    out, _ = _run(inputs)
    return out


# revision 12
# speedup vs baseline: 1.1278x; 1.0249x over previous
"""Trainium2 Bass kernel for a 6-layer GRU stack (T=1, B=1, H=2048) + LayerNorm +
Linear with residual, tensor-parallel over 8 NeuronCores.

Contract: kernel(**inputs) takes the FULL unsharded inputs (as produced by the
reference setup_inputs) and returns the full outputs (pred [1,2048],
new_hidden [6,1,2048]).

Strategy (hardcoded for L=6, H=2048, 8 cores):
  - Shard each layer's 3H=6144 gate rows: core k owns rows [k*256,(k+1)*256) of
    each of the r/z/n gate blocks (768 rows/core/matrix).
  - Host-side: slice + transpose weights to W^T, chunk the contraction dim into
    16 chunks of 128 (chunk n holds contraction elements {16p+n}), cast bf16.
  - Device: GEMV = 16 matmuls with a [128,1] slice of the hidden-state
    stationary tile ([128,16] = h.reshape(128,16)) against [128,768] moving
    weight tiles, accumulating in PSUM.  Biases folded in via 1-contraction
    matmuls against a ones stationary.
  - Gate nonlinearities on [1,768]-ish tiles (ScalarE sigmoid/tanh, VectorE TT).
  - Per layer: AllGather of the 256-element h_new chunk (bf16) via DRAM bounce
    to rebuild the full 2048 hidden state on every core.
  - Tail: LayerNorm computed redundantly on every core from the final gathered
    h (partition-sum / broadcast via ones-matmuls), then a row-sharded Linear
    (256 out rows/core) + residual; host concatenates the 8 chunks.
"""
import numpy as np
import ml_dtypes

L = 6
H = 2048
NCORES = 8
S = H // NCORES          # 256  per-core hidden chunk
G = 3 * S                # 768  gate rows per core
NCH = H // 128           # 16   contraction chunks
LN_EPS = 1e-5

BF16 = ml_dtypes.bfloat16

_cache = {}


def _build_nc():
    import concourse.bacc as bacc
    import concourse.mybir as mybir
    import concourse.tile as tile

    dt = mybir.dt
    AF = mybir.ActivationFunctionType
    ALU = mybir.AluOpType

    nc = bacc.Bacc("TRN2", target_bir_lowering=False, debug=False,
                   num_devices=NCORES)

    wi = nc.dram_tensor("wi", [L, 128, NCH, G], dt.bfloat16, kind="ExternalInput")
    wh = nc.dram_tensor("wh", [L, 128, NCH, G], dt.bfloat16, kind="ExternalInput")
    lw = nc.dram_tensor("lw", [128, NCH, S], dt.bfloat16, kind="ExternalInput")
    xst = nc.dram_tensor("xst", [128, NCH], dt.bfloat16, kind="ExternalInput")
    hst = nc.dram_tensor("hst", [128, L, NCH], dt.bfloat16, kind="ExternalInput")
    hch = nc.dram_tensor("hch", [1, L * S], dt.float32, kind="ExternalInput")
    brz = nc.dram_tensor("brz", [1, L * 2 * S], dt.bfloat16, kind="ExternalInput")
    bin_ = nc.dram_tensor("bin", [1, L * S], dt.bfloat16, kind="ExternalInput")
    bhn = nc.dram_tensor("bhn", [1, L * S], dt.bfloat16, kind="ExternalInput")
    lb = nc.dram_tensor("lb", [1, S], dt.float32, kind="ExternalInput")
    xres = nc.dram_tensor("xres", [1, S], dt.float32, kind="ExternalInput")
    gam = nc.dram_tensor("gam", [128, NCH], dt.float32, kind="ExternalInput")
    bet = nc.dram_tensor("bet", [128, NCH], dt.float32, kind="ExternalInput")
    ones = nc.dram_tensor("ones", [128, 128], dt.float32, kind="ExternalInput")

    out_h = nc.dram_tensor("out_h", [L, S], dt.float32, kind="ExternalOutput")
    out_p = nc.dram_tensor("out_p", [1, S], dt.float32, kind="ExternalOutput")

    RG = [list(range(NCORES))]

    with tile.TileContext(nc) as tc:
        with tc.tile_pool(name="wpool", bufs=2) as wpool, \
             tc.tile_pool(name="spool", bufs=2) as spool, \
             tc.tile_pool(name="cpool", bufs=1) as cpool, \
             tc.tile_pool(name="psum", bufs=2, space="PSUM") as pp, \
             tc.tile_pool(name="dram", bufs=2, space="DRAM") as dpool:

            # ---- constants / small inputs (loaded once) ----
            xst_sb = cpool.tile([128, NCH], dt.bfloat16)
            nc.sync.dma_start(out=xst_sb[:], in_=xst.ap())
            hst_sb = cpool.tile([128, L, NCH], dt.bfloat16)
            nc.sync.dma_start(out=hst_sb[:], in_=hst.ap())
            hch_sb = cpool.tile([1, L * S], dt.float32)
            nc.sync.dma_start(out=hch_sb[:], in_=hch.ap())
            brz_sb = cpool.tile([1, L * 2 * S], dt.bfloat16)
            nc.sync.dma_start(out=brz_sb[:], in_=brz.ap())
            bin_sb = cpool.tile([1, L * S], dt.bfloat16)
            nc.sync.dma_start(out=bin_sb[:], in_=bin_.ap())
            bhn_sb = cpool.tile([1, L * S], dt.bfloat16)
            nc.sync.dma_start(out=bhn_sb[:], in_=bhn.ap())
            lb_sb = cpool.tile([1, S], dt.float32)
            nc.sync.dma_start(out=lb_sb[:], in_=lb.ap())
            xres_sb = cpool.tile([1, S], dt.float32)
            nc.sync.dma_start(out=xres_sb[:], in_=xres.ap())
            gam_sb = cpool.tile([128, NCH], dt.float32)
            nc.sync.dma_start(out=gam_sb[:], in_=gam.ap())
            bet_sb = cpool.tile([128, NCH], dt.float32)
            nc.sync.dma_start(out=bet_sb[:], in_=bet.ap())
            ones_sb = cpool.tile([128, 128], dt.float32)
            nc.sync.dma_start(out=ones_sb[:], in_=ones.ap())
            onesb_sb = cpool.tile([1, 16], dt.bfloat16)
            nc.gpsimd.dma_start(out=onesb_sb[:], in_=ones.ap()[0:1, 0:16])

            one1 = onesb_sb[0:1, 0:1]         # [1,1] bf16 bias-matmul stationary
            one1f = ones_sb[0:1, 0:1]         # [1,1] fp32
            ones_col = ones_sb[:, 0:1]        # [128,1] partition-sum stationary
            ones_row = ones_sb[0:1, :]        # [1,128] broadcast stationary

            prev_st = xst_sb                  # layer-0 gi stationary = x_t

            psA = {}   # [1,512] psum: r|z preactivation (gi+gh+biases)
            psB = {}   # [1,512] psum: gi_n (cols 0:S) | gh_n (cols S:2S)
            wis = {}
            whs = {}

            # warmup collective: absorbs ncfw first-call cost + aligns cores
            wu_in = dpool.tile([1, 8], dt.float32, tag="wuin")
            nc.sync.dma_start(out=wu_in[:], in_=ones.ap()[0:1, 0:8])
            wu_out = dpool.tile([NCORES, 8], dt.float32, tag="wuout",
                                addr_space="Shared")
            nc.gpsimd.collective_compute(
                "AllGather", ALU.bypass, replica_groups=RG,
                ins=[wu_in[:]], outs=[wu_out[:]],
            )

            def emit_whdma(l):
                wh_sb = wpool.tile([128, NCH, G], dt.bfloat16, tag="wh", bufs=4)
                nc.gpsimd.dma_start(out=wh_sb[:], in_=wh.ap()[l])
                whs[l] = wh_sb

            def emit_widma(l):
                wi_sb = wpool.tile([128, NCH, G], dt.bfloat16, tag="wi", bufs=3)
                nc.gpsimd.dma_start(out=wi_sb[:], in_=wi.ap()[l])
                wis[l] = wi_sb

            def emit_gh(l):
                wh_sb = whs[l]
                A = pp.tile([1, 2 * S], dt.float32, tag="A", bufs=4)
                B = pp.tile([1, 2 * S], dt.float32, tag="B", bufs=4)
                psA[l], psB[l] = A, B
                nc.tensor.matmul(A[:], one1,
                                 brz_sb[:, l * 2 * S:(l + 1) * 2 * S],
                                 start=True, stop=False)
                nc.tensor.matmul(B[:, 0:S], one1, bin_sb[:, l * S:(l + 1) * S],
                                 start=True, stop=False)
                nc.tensor.matmul(B[:, S:2 * S], one1, bhn_sb[:, l * S:(l + 1) * S],
                                 start=True, stop=False)
                for n in range(NCH):
                    st = hst_sb[:, l, n:n + 1]
                    nc.tensor.matmul(A[:], st, wh_sb[:, n, 0:2 * S],
                                     start=False, stop=False)
                    nc.tensor.matmul(B[:, S:2 * S], st, wh_sb[:, n, 2 * S:G],
                                     start=False, stop=(n == NCH - 1))

            def emit_gi(l):
                A, B = psA[l], psB[l]
                wi_sb = wis[l]
                for n in range(NCH):       # A first: sigmoid starts earlier
                    st = prev_st[:, n:n + 1]
                    nc.tensor.matmul(A[:], st, wi_sb[:, n, 0:2 * S],
                                     start=False, stop=(n == NCH - 1))
                for n in range(NCH):
                    st = prev_st[:, n:n + 1]
                    nc.tensor.matmul(B[:, 0:S], st, wi_sb[:, n, 2 * S:G],
                                     start=False, stop=(n == NCH - 1))

            # ---- prologue: preload gh weights; PE chews warmup + gh_0..3 ----
            for l in range(4):
                emit_whdma(l)
            for l in range(3):
                emit_widma(l)
            ps_warm = pp.tile([1, NCH], dt.float32, tag="A", bufs=4)
            for i in range(24):
                nc.tensor.matmul(ps_warm[:], xst_sb[:, 0:1], xst_sb[:, :],
                                 start=(i == 0), stop=(i == 23))
            emit_whdma(4)
            for l in range(4):
                emit_gh(l)

            for l in range(L):
                emit_gi(l)
                if l + 3 < L:
                    emit_widma(l + 3)
                if l == 3:
                    lw_sb = wpool.tile([128, NCH, S], dt.bfloat16, tag="wi",
                                       bufs=3)
                    nc.sync.dma_start(out=lw_sb[:], in_=lw.ap())
                if l == 0:
                    emit_whdma(5)
                if l + 4 < L:
                    emit_gh(l + 4)
                A, B = psA[l], psB[l]

                # ---- gates ----
                sig = spool.tile([1, 2 * S], dt.float32, tag="sig")
                nc.scalar.activation(sig[:], A[:], AF.Sigmoid)
                t_ = spool.tile([1, S], dt.float32, tag="t")
                nc.vector.tensor_mul(t_[:], sig[:, 0:S], B[:, S:2 * S])
                u_ = spool.tile([1, S], dt.float32, tag="u")
                nc.vector.tensor_add(u_[:], B[:, 0:S], t_[:])
                nn_ = spool.tile([1, S], dt.float32, tag="nn")
                nc.scalar.activation(nn_[:], u_[:], AF.Tanh)
                d_ = spool.tile([1, S], dt.float32, tag="d")
                nc.vector.tensor_sub(d_[:], hch_sb[:, l * S:(l + 1) * S], nn_[:])
                zd = spool.tile([1, S], dt.float32, tag="zd")
                nc.vector.tensor_mul(zd[:], sig[:, S:2 * S], d_[:])
                hnew = spool.tile([1, S], dt.bfloat16, tag="hnew")
                nc.vector.tensor_add(hnew[:], nn_[:], zd[:])

                # fp32 hidden-state output (off critical path, SWDGE cast)
                nc.gpsimd.dma_start(out=out_h.ap()[l:l + 1, :], in_=hnew[:])

                # ---- exchange: AllGather, reload as stationary ----
                ag_in = dpool.tile([1, S], dt.bfloat16, tag="agin")
                nc.sync.dma_start(out=ag_in[:], in_=hnew[:])
                ag_out = dpool.tile([128, NCH], dt.bfloat16, tag="agout",
                                    addr_space="Shared")
                nc.gpsimd.collective_compute(
                    "AllGather", ALU.bypass, replica_groups=RG,
                    ins=[ag_in[:]], outs=[ag_out[:]],
                )
                h_st = spool.tile([128, NCH], dt.bfloat16, tag="hstat")
                nc.sync.dma_start(out=h_st[:], in_=ag_out[:])
                prev_st = h_st

            # ---- LayerNorm on the gathered top hidden state ----
            stats = spool.tile([128, 2], dt.float32)
            nc.vector.reduce_sum(stats[:, 0:1], prev_st[:], axis=mybir.AxisListType.X)
            sq = spool.tile([128, NCH], dt.float32)
            nc.scalar.activation(sq[:], prev_st[:], AF.Square,
                                 accum_out=stats[:, 1:2])
            ps_stats = pp.tile([1, 2], dt.float32, tag="A", bufs=4)
            nc.tensor.matmul(ps_stats[:], ones_col, stats[:], start=True, stop=True)
            mean2 = spool.tile([1, 2], dt.float32)
            nc.vector.tensor_scalar_mul(mean2[:], ps_stats[:], 1.0 / H)
            musd = spool.tile([1, 2], dt.float32)
            nc.vector.tensor_copy(musd[:, 0:1], mean2[:, 0:1])
            mu2 = spool.tile([1, 1], dt.float32)
            nc.vector.tensor_mul(mu2[:], mean2[:, 0:1], mean2[:, 0:1])
            var = spool.tile([1, 1], dt.float32)
            nc.vector.tensor_sub(var[:], mean2[:, 1:2], mu2[:])
            nc.vector.tensor_scalar_add(var[:], var[:], LN_EPS)
            std = spool.tile([1, 1], dt.float32)
            nc.scalar.activation(std[:], var[:], AF.Sqrt)
            nc.vector.reciprocal(musd[:, 1:2], std[:])
            ps_bc = pp.tile([128, 2], dt.float32, tag="B", bufs=4)
            nc.tensor.matmul(ps_bc[:], ones_row, musd[:], start=True, stop=True)
            bc = spool.tile([128, 2], dt.float32)
            nc.vector.tensor_copy(bc[:], ps_bc[:])
            # y = ((x - mu) * rstd) * gamma + beta
            y = spool.tile([128, NCH], dt.float32)
            nc.vector.tensor_scalar(y[:], prev_st[:], bc[:, 0:1], bc[:, 1:2],
                                    op0=ALU.subtract, op1=ALU.mult)
            nc.vector.tensor_mul(y[:], y[:], gam_sb[:])
            nc.vector.tensor_add(y[:], y[:], bet_sb[:])
            ln_bf = spool.tile([128, NCH], dt.bfloat16)
            nc.vector.tensor_copy(ln_bf[:], y[:])

            # ---- Linear (row shard) + residual ----
            ps_pred = pp.tile([1, S], dt.float32, tag="A", bufs=4)
            nc.tensor.matmul(ps_pred[:], one1f, lb_sb[:], start=True, stop=False)
            for n in range(NCH):
                nc.tensor.matmul(ps_pred[:], ln_bf[:, n:n + 1], lw_sb[:, n, :],
                                 start=False, stop=(n == NCH - 1))
            pred = spool.tile([1, S], dt.float32)
            nc.vector.tensor_add(pred[:], ps_pred[:], xres_sb[:])
            nc.sync.dma_start(out=out_p.ap(), in_=pred[:])

    nc.compile()
    return nc


def _prep_inputs(inputs):
    """Full inputs -> list of 8 per-core in_maps (host-side shard/transpose)."""
    x = np.asarray(inputs["x"], np.float32)
    hidden = np.asarray(inputs["hidden"], np.float32)
    w_ih = np.asarray(inputs["w_ih"], np.float32)
    w_hh = np.asarray(inputs["w_hh"], np.float32)
    b_ih = np.asarray(inputs["b_ih"], np.float32)
    b_hh = np.asarray(inputs["b_hh"], np.float32)
    ln_gamma = np.asarray(inputs["ln_gamma"], np.float32)
    ln_beta = np.asarray(inputs["ln_beta"], np.float32)
    lin_w = np.asarray(inputs["lin_w"], np.float32)
    lin_b = np.asarray(inputs["lin_b"], np.float32)

    x_t = x[0, -1]                        # [H]
    hid = hidden[:, 0]                    # [L, H]

    def shard_gates(w):
        # [L,3H,H] -> [8, L, NCH, 128, G]  (W^T, contraction-chunked)
        a = w.astype(BF16).reshape(L, 3, NCORES, S, H)
        a = a.transpose(2, 0, 1, 3, 4).reshape(NCORES, L, G, H)
        a = a.transpose(0, 1, 3, 2)                      # [8, L, H, G]
        a = a.reshape(NCORES, L, 128, NCH, G)            # partition-major
        return np.ascontiguousarray(a)                   # [8, L, 128, NCH, G]

    wi8 = shard_gates(w_ih)
    wh8 = shard_gates(w_hh)

    lwt = lin_w.astype(BF16).reshape(NCORES, S, H)        # row shards
    lwt = lwt.transpose(0, 2, 1)                          # [8, H, S]
    lw8 = np.ascontiguousarray(lwt.reshape(NCORES, 128, NCH, S))  # [8,128,NCH,S]

    xst = np.ascontiguousarray(x_t.astype(BF16).reshape(128, NCH))
    hst = np.ascontiguousarray(hid.astype(BF16).reshape(L, 128, NCH).transpose(1, 0, 2))
    gamt = np.ascontiguousarray(ln_gamma.reshape(128, NCH))
    bett = np.ascontiguousarray(ln_beta.reshape(128, NCH))
    onesm = np.ones((128, 128), np.float32)

    bi = b_ih.reshape(L, 3, NCORES, S)
    bh = b_hh.reshape(L, 3, NCORES, S)

    in_maps = []
    for k in range(NCORES):
        brz = (bi[:, 0:2, k] + bh[:, 0:2, k]).reshape(1, L * 2 * S).astype(BF16)
        bin_ = bi[:, 2, k].reshape(1, L * S).astype(BF16)
        bhn = bh[:, 2, k].reshape(1, L * S).astype(BF16)
        in_maps.append({
            "wi": wi8[k],
            "wh": wh8[k],
            "lw": lw8[k],
            "xst": xst,
            "hst": hst,
            "hch": np.ascontiguousarray(hid[:, k * S:(k + 1) * S]).reshape(1, L * S),
            "brz": np.ascontiguousarray(brz),
            "bin": np.ascontiguousarray(bin_),
            "bhn": np.ascontiguousarray(bhn),
            "lb": lin_b[k * S:(k + 1) * S].reshape(1, S).copy(),
            "xres": x_t[k * S:(k + 1) * S].reshape(1, S).copy(),
            "gam": gamt,
            "bet": bett,
            "ones": onesm,
        })
    return in_maps


def _run(inputs, trace=False, tmpdir=None):
    import concourse.bass_utils as bass_utils
    if "nc" not in _cache:
        _cache["nc"] = _build_nc()
    nc = _cache["nc"]
    in_maps = _prep_inputs(inputs)
    res = bass_utils.run_bass_kernel_spmd(
        nc, in_maps, core_ids=list(range(NCORES)), trace=trace, tmpdir=tmpdir)
    pred = np.concatenate([res.results[k]["out_p"][0] for k in range(NCORES)])
    new_hidden = np.stack(
        [np.concatenate([res.results[k]["out_h"][l] for k in range(NCORES)])
         for l in range(L)])[:, None, :]
    return (pred[None, :].astype(np.float32),
            new_hidden.astype(np.float32)), res


def kernel(**inputs):
    out, _ = _run(inputs)
    return out
